# revision 1
# baseline (speedup 1.0000x reference)
"""Trainium2 Bass kernel for DCTLAVISBlip dc_transform (DCT -> truncate -> IDCT).

Strategy
--------
reference(x) computes, for x [B=64, T=576, C=1024] f32:
  1. y = DCT_II(x) along tokens:  y[b] = M @ x[b]            (M = [576,576] ortho DCT)
  2. v = |mean_{b,c} y|  -> threshold = quantile(v, 0.8) -> last_index -> L
  3. x_dct_trunc = y[:, :L, :]                               (f32 output)
  4. state = IDCT_L(x_dct_trunc) = Mi_pad^T @ y  -> f16      (Mi = [L,L] ortho DCT)

Because mean commutes with the linear DCT, v = |M @ mean_{b,c}(x)| is computed
on the host from a length-576 vector -- no device round trip. The IDCT is fused
into a second weight block P = Mi_pad^T @ M, so the device does one stacked
matmul W @ x[b] with W = [M; P] [1152, 576] per batch, data-parallel over B
across 8 NeuronCores (8 batches each).

Device kernel (per core, 8 batches): single-pass fp16 matmuls (fp32 PSUM
accumulation), 4-wide PSUM groups so consecutive matmuls reuse the stationary
weight, the K=64 contraction remainder row-packed pairwise onto disjoint PE
row groups (tile_position), PE pre-warmed with dummy matmuls during the input
DMA head, outputs shipped as f16 (host upcasts y to f32), input DMAs issued
in first-use order on sync queues, output DMAs on gpsimd for the first two
groups (while inputs stream) then sync for the rest. Outputs stage both
n-halves into one full-width f16 tile (2KB-contiguous DMA rows, half the DMA
count), with both copies of a tile on one engine (alternating engines per
tile) so no tile is cross-engine serialized. Measured ~170.7 us on hardware
vs a ~140 us PE-streaming floor; trace shows ~6 us fixed preamble, ~3 us PE
gaps, ~6 us fixed end barrier. Accuracy ~7e-4 relative (gate ~2e-2),
dominated by the fp16 casts.
"""

import numpy as np

B, T, C = 64, 576, 1024
NCORES = 8
BPC = B // NCORES            # batches per core
W_OUT = 2 * T                # stacked output rows: [M; P]
Q = 0.8

K_TILES = [(0, 128), (128, 128), (256, 128), (384, 128), (512, 64)]
M_TILES = [(i * 128, 128) for i in range(W_OUT // 128)]   # 9 tiles over 1152
N_TILES = [(0, 512), (512, 512)]

_CACHED = {}


def _dct_mat(N):
    n = np.arange(N)
    Mm = np.cos(np.pi * (2 * n[None, :] + 1) * n[:, None] / (2 * N))
    s = np.full(N, np.sqrt(2.0 / N))
    s[0] = np.sqrt(1.0 / N)
    return s[:, None] * Mm          # float64


def _build_nc():
    import concourse.bacc as bacc
    import concourse.mybir as mybir
    import concourse.tile as tile

    f16 = mybir.dt.float16
    f32 = mybir.dt.float32

    nc = bacc.Bacc("TRN2", target_bir_lowering=False, debug=False,
                   num_devices=NCORES)
    xh = nc.dram_tensor("xh", [BPC, T, C], f16, kind="ExternalInput")
    wt = nc.dram_tensor("wt", [T, W_OUT], f16, kind="ExternalInput")
    # y (the f32 x_dct output) ships as f16 to halve output DMA; the host
    # upcasts. Quantization adds ~2.4e-4 relative, well inside tolerance.
    y = nc.dram_tensor("y", [BPC, T, C], f16, kind="ExternalOutput")
    st = nc.dram_tensor("st", [BPC, T, C], f16, kind="ExternalOutput")

    # (b, n) pairs in groups of 4 sharing one PSUM quad; pairs ordered so a
    # group only needs two batches' x tiles (prefetch-friendly). Batches in a
    # group form an (even, odd) pair so the K=64 remainder k-tile can be
    # row-packed: both batches' remainder rows live in one 128-partition tile
    # and run as two concurrent matmuls on disjoint PE row groups.
    pairs = [(b, n) for b in range(BPC) for n in range(len(N_TILES))]
    groups = [pairs[i:i + 4] for i in range(0, len(pairs), 4)]
    NKF = 4                       # full 128-row k-tiles; k-tile 4 is the 64-row rest
    K4 = K_TILES[NKF][0]          # 512

    with tile.TileContext(nc) as tc:
        with (
            tc.tile_pool(name="wpool", bufs=1) as wpool,
            tc.tile_pool(name="xpool", bufs=1) as xpool,
            tc.tile_pool(name="ysb", bufs=10) as ypool,
            tc.tile_pool(name="ssb", bufs=10) as spool,
            tc.tile_pool(name="ps", bufs=8, space="PSUM") as ps,
        ):
            # Engine warmup during the input-DMA head (no DMA deps): dummy
            # matmuls flip the PE HAM clock gate to 8/8, and dummy copies
            # take the Scalar/Vector engines' cold-start penalty off the
            # PSUM-drain critical path.
            wz = wpool.tile([128, 128], f16, tag="wz")
            wd = wpool.tile([128, 128], f16, tag="wd")
            nc.gpsimd.memset(wz[:], 0.0)
            pwarm = ps.tile([128, 128], f32, tag="pt", name="pt")
            for _ in range(36):
                nc.tensor.matmul(pwarm[:], wz[:], wz[:], start=True, stop=True)
            # Issue input DMAs in first-use order so the PE can start as soon
            # as (w0, x[b0,0], x[b1,0]) land instead of after the whole load.
            wts = [None] * NKF
            xts = {}
            x4 = {}
            for i in range(NKF):
                k0, kk = K_TILES[i]
                t_ = wpool.tile([kk, W_OUT], f16, tag=f"w{i}", name=f"w{i}")
                nc.sync.dma_start(t_[:], wt[k0:k0 + kk, :])
                wts[i] = t_
                for bb in (0, 1):
                    tx = xpool.tile([kk, C], f16, tag=f"x{bb}_{i}", name=f"x{bb}_{i}")
                    nc.sync.dma_start(tx[:], xh[bb, k0:k0 + kk, :])
                    xts[(bb, i)] = tx
            # K=64 remainder weights, duplicated into both partition halves
            w4d = wpool.tile([128, W_OUT], f16, tag="w4d")
            nc.sync.dma_start(w4d[0:64, :], wt[K4:T, :])
            nc.sync.dma_start(w4d[64:128, :], wt[K4:T, :])
            t4 = xpool.tile([128, C], f16, tag="x4_0", name="x4_0")
            nc.sync.dma_start(t4[0:64, :], xh[0, K4:T, :])
            nc.sync.dma_start(t4[64:128, :], xh[1, K4:T, :])
            x4[0] = t4

            for b in range(2, BPC, 2):
                for bb in (b, b + 1):
                    for i in range(NKF):
                        k0, kk = K_TILES[i]
                        t_ = xpool.tile([kk, C], f16, tag=f"x{bb}_{i}", name=f"x{bb}_{i}")
                        nc.sync.dma_start(t_[:], xh[bb, k0:k0 + kk, :])
                        xts[(bb, i)] = t_
                # both batches' K=64 remainder rows share one 128-tall tile
                t4 = xpool.tile([128, C], f16, tag=f"x4_{b}", name=f"x4_{b}")
                nc.sync.dma_start(t4[0:64, :], xh[b, K4:T, :])
                nc.sync.dma_start(t4[64:128, :], xh[b + 1, K4:T, :])
                x4[b] = t4

            for gi, g in enumerate(groups):
                gb = g[0][0]                      # even batch of this group
                oeng = nc.gpsimd if gi < 2 else nc.sync
                for mi, (m0, mm) in enumerate(M_TILES):
                    pts = []
                    for (b, n) in g:
                        pts.append(ps.tile([128, 512], f32, tag="pt", name="pt"))
                    for ki in range(NKF):
                        for pi, (b, n) in enumerate(g):
                            n0, nn = N_TILES[n]
                            nc.tensor.matmul(
                                pts[pi][:],
                                wts[ki][:, m0:m0 + mm],
                                xts[(b, ki)][:, n0:n0 + nn],
                                start=(ki == 0),
                                stop=False,
                            )
                    # K=64 remainder: row-packed concurrent pairs
                    for n in range(len(N_TILES)):
                        n0, nn = N_TILES[n]
                        for half, pi in ((0, n), (1, 2 + n)):
                            nc.tensor.matmul(
                                pts[pi][:],
                                w4d[64 * half:64 * half + 64, m0:m0 + mm],
                                x4[gb][64 * half:64 * half + 64, n0:n0 + nn],
                                start=False,
                                stop=True,
                                tile_position=(64 * half, 0),
                            )
                    # drain psum -> sbuf -> dram. Both n-halves of one batch
                    # stage into a single full-width tile (2KB-contiguous DMA
                    # rows, half the DMA count); both copies of a tile run on
                    # ONE engine so the tile is never cross-engine serialized,
                    # with engines alternating per tile for balance.
                    for bi, b in enumerate((gb, gb + 1)):
                        p0, p1 = 2 * bi, 2 * bi + 1     # pair idx for n0, n1
                        if m0 + mm <= T:            # pure y tile
                            ot = ypool.tile([128, 1024], f16, tag="yo")
                            if bi == 0:
                                nc.vector.tensor_copy(ot[:, 0:512], pts[p0][:])
                                nc.vector.tensor_copy(ot[:, 512:1024], pts[p1][:])
                            else:
                                nc.scalar.copy(ot[:, 0:512], pts[p0][:])
                                nc.scalar.copy(ot[:, 512:1024], pts[p1][:])
                            oeng.dma_start(y[b, m0:m0 + mm, :], ot[:])
                        elif m0 >= T:               # pure state tile
                            ot = spool.tile([128, 1024], f16, tag="so")
                            if bi == 0:
                                nc.scalar.copy(ot[:, 0:512], pts[p0][:])
                                nc.scalar.copy(ot[:, 512:1024], pts[p1][:])
                            else:
                                nc.vector.tensor_copy(ot[:, 0:512], pts[p0][:])
                                nc.vector.tensor_copy(ot[:, 512:1024], pts[p1][:])
                            oeng.dma_start(
                                st[b, m0 - T:m0 - T + mm, :], ot[:])
                        else:                       # straddles y/state boundary
                            half = T - m0           # = 64
                            oy = ypool.tile([64, 1024], f16, tag="yh")
                            os_ = spool.tile([64, 1024], f16, tag="sh")
                            nc.vector.tensor_copy(oy[:, 0:512], pts[p0][0:half, :])
                            nc.vector.tensor_copy(oy[:, 512:1024], pts[p1][0:half, :])
                            nc.scalar.copy(os_[:, 0:512], pts[p0][half:128, :])
                            nc.scalar.copy(os_[:, 512:1024], pts[p1][half:128, :])
                            oeng.dma_start(y[b, m0:T, :], oy[:])
                            oeng.dma_start(
                                st[b, 0:m0 + mm - T, :], os_[:])
    nc.finalize()
    return nc


def _get_nc():
    if "nc" not in _CACHED:
        _CACHED["nc"] = _build_nc()
    return _CACHED["nc"]


def _ensure_trace_hook_safe():
    """If BASS_TRACE is set in the environment, run_bass_kernel_spmd imports
    antenv.axon_hooks, which may not exist. Install a working ctypes-based
    shim when possible, else disable tracing so the run cannot crash."""
    import os
    import sys
    import types

    if not os.environ.get("BASS_TRACE"):
        return
    try:
        import antenv.axon_hooks  # noqa: F401
        return
    except ImportError:
        pass
    try:
        from trn_agent_boot.trn_boot import _ntff_profile_via_ctypes
        hooks = types.ModuleType("antenv.axon_hooks")
        hook = _ntff_profile_via_ctypes("/opt/axon/libaxon_pjrt.so")
        hooks.get_axon_ntff_profile_hook = lambda: hook
        hooks.set_axon_ntff_profile_hook = lambda h: None
        sys.modules["antenv.axon_hooks"] = hooks
    except Exception:
        os.environ["BASS_NEVER_TRACE"] = "1"


def kernel(x: np.ndarray):
    from concourse.bass_utils import run_bass_kernel_spmd

    _ensure_trace_hook_safe()
    x = np.ascontiguousarray(np.asarray(x, dtype=np.float32))
    assert x.shape == (B, T, C)

    # ---- host: data-dependent truncation length L (tiny, exact math) ----
    M64 = _dct_mat(T)
    xbar = x.astype(np.float64).mean(axis=(0, 2))
    v = np.abs(M64 @ xbar)
    thr = np.abs(np.quantile(v, Q))
    idxs = np.where(v > thr)[0]
    last_index = int(idxs[-1]) if idxs.size > 0 else -1
    L = last_index if last_index >= 0 else T - 1   # len of y[:, :last_index, :]

    # ---- host: stacked weight [M; P],  P = Mi_pad^T @ M ----
    if L > 0:
        Mi = _dct_mat(L)
        P = Mi.T @ M64[:L, :]
    else:
        P = np.zeros((0, T))
    P_full = np.zeros((T, T))
    P_full[:P.shape[0], :] = P
    Wfull = np.concatenate([M64, P_full], axis=0)          # [1152, 576]
    wt16 = np.ascontiguousarray(Wfull.T).astype(np.float16)  # [576, 1152]

    xh = x.astype(np.float16)

    nc = _get_nc()
    in_maps = [
        {"xh": np.ascontiguousarray(xh[i * BPC:(i + 1) * BPC]), "wt": wt16}
        for i in range(NCORES)
    ]
    res = run_bass_kernel_spmd(nc, in_maps, list(range(NCORES)))
    _CACHED["last_exec_time_ns"] = res.exec_time_ns

    y = np.concatenate([res.results[i]["y"] for i in range(NCORES)], axis=0)
    stt = np.concatenate([res.results[i]["st"] for i in range(NCORES)], axis=0)

    x_dct_trunc = y[:, :L, :].astype(np.float32)
    state = np.ascontiguousarray(stt[:, :L, :])
    return state, x_dct_trunc

